# revision 1
# baseline (speedup 1.0000x reference)
"""MoE (63 routed experts, top-7, 1 shared expert) Trainium2 Bass kernel.

Strategy (expert parallelism, per sharding hint):
  - Host: router matmul + softmax + top-k (tiny: 0.7 GFLOP vs 220 GFLOP of
    expert FFNs), token gather per expert.
  - Device (8 NeuronCores, SPMD): each core runs 9 "units" of identical
    shape: 8 routed-expert slots (64 slots globally = 63 experts + 1
    overflow slot) and 1 shared-expert slot over a 1/8 token slice.
    Each unit: h = gelu(XeT^T @ W1 + b1); y = gate * (h @ W2), with
    full-rate matmuls (float32r or bf16), GELU fused into the PSUM
    eviction on the scalar engine, gating fused into the PSUM eviction on
    the vector engine.  Weights are host-pretiled into chunk-contiguous
    layout so every DMA is a flat [128 x bytes] block.
  - Host: scatter-add gated expert outputs (+ gate*b2), add shared out,
    bias and residual.

Experts are assigned to slots by descending load rank with static per-unit
token capacities (CAPS); both matmul layers' free dim is the capacity, so
PE cost tracks actual expert load.  Overload spills into the spare 64th
slot and, beyond that, to an exact host-side FFN for the few excess
tokens.  Gating and b2 are applied on the host during the scatter.
"""

import os

import numpy as np

B, S, HID = 2, 2048, 1280
E = 63
I = 1280
TOP_K = 7
NCORES = 8
UNITS = 9          # 8 expert slots + 1 shared-expert slot
C = 512            # token capacity per expert slot
CM = C // 128      # 4
KO = HID // 128    # 10 contraction chunks
T = B * S          # 4096
TSH = T // NCORES  # 512 shared-expert tokens per core

W1CW = 256          # w1 chunk width along I (2 lhsT column groups)
W2CW = 256          # w2 chunk width along H (2 lhsT column groups)
N_W1C = I // W1CW   # 5
N_W2C = HID // W2CW  # 5

# Per-unit-index token capacities. Experts are assigned to slots by load
# rank (rank r -> core r%8, unit r//8), so unit j only ever sees the j-th
# bucket of the descending load distribution; caps cover the bucket maxima
# of any near-uniform routing with margin. Uncovered overflow goes to the
# spare slot 63 and, beyond that, to an exact host fallback.
CAPS = [512, 500, 484, 472, 460, 448, 440, 420, C]   # unit 8 = shared

# "f32r": fp32 data, full-rate float32r matmuls (most accurate).
# "bf16": bf16 weights+activations, fp32 accumulate (halves DMA traffic).
# "fp16": like bf16 but 4x finer mantissa; all values here are well within
#         fp16 range, so this is strictly more accurate at the same speed.
WORK_DTYPE = os.environ.get("MOE_WDT", "fp16")

_cache = {}


def _build_nc(wdt):
    import concourse.mybir as mybir
    import concourse.tile as tile
    from concourse import bacc

    f32 = mybir.dt.float32
    GELU = mybir.ActivationFunctionType.Gelu
    if wdt == "f32r":
        mdt = mybir.dt.float32r
        ddt = f32    # dram dtype for weight/activation tensors
        bufs = dict(xu=2, h1=2, w1c=3, w2c=3, ou=2)
    else:
        mdt = mybir.dt.float16 if wdt == "fp16" else mybir.dt.bfloat16
        ddt = mdt
        bufs = dict(xu=3, h1=3, w1c=4, w2c=4, ou=2)

    nc = bacc.Bacc(None, target_bir_lowering=False)

    xg_d = nc.dram_tensor("xg", [UNITS, 128, KO, C], ddt, kind="ExternalInput")
    w1_d = nc.dram_tensor("w1", [UNITS, N_W1C, 128, KO, W1CW], ddt,
                          kind="ExternalInput")
    b1_d = nc.dram_tensor("b1", [UNITS, 128, KO], f32, kind="ExternalInput")
    w2_d = nc.dram_tensor("w2", [UNITS, N_W2C, 128, KO, W2CW], ddt,
                          kind="ExternalInput")
    # transposed output: out[u, p, hk, c] = y[token c, h = hk*128+p]
    out_d = nc.dram_tensor("out", [UNITS, 128, KO, C], f32, kind="ExternalOutput")

    def cast(ap):
        return ap.bitcast(mdt) if wdt == "f32r" else ap

    with tile.TileContext(nc) as tc:
        with tc.tile_pool(name="xg_p", bufs=bufs["xu"]) as xg_p, \
             tc.tile_pool(name="h1_p", bufs=bufs["h1"]) as h1_p, \
             tc.tile_pool(name="w1_p", bufs=bufs["w1c"]) as w1_p, \
             tc.tile_pool(name="w2_p", bufs=bufs["w2c"]) as w2_p, \
             tc.tile_pool(name="out_p", bufs=bufs["ou"]) as out_p, \
             tc.tile_pool(name="sm_p", bufs=3) as sm_p, \
             tc.tile_pool(name="ps1_p", bufs=3, space="PSUM") as ps1_p, \
             tc.tile_pool(name="ps2_p", bufs=4, space="PSUM") as ps2_p:

            for u in range(UNITS):
                CAP = CAPS[u]
                w1cs = {}
                # first w1 chunk ahead of everything else the unit needs
                w1cs[0] = w1_p.tile([128, KO, W1CW], mdt, tag="w1c", name="w1c")
                nc.sync.dma_start(w1cs[0][:], cast(w1_d[u, 0]))
                xu = xg_p.tile([128, KO, C], mdt, tag="xu")
                # split halves so the first matmuls can start sooner
                nc.sync.dma_start(xu[:, :KO // 2, :CAP],
                                  cast(xg_d[u, :, :KO // 2, :CAP]))
                nc.sync.dma_start(xu[:, KO // 2:, :CAP],
                                  cast(xg_d[u, :, KO // 2:, :CAP]))
                b1u = sm_p.tile([128, KO], f32, tag="b1u")
                nc.sync.dma_start(b1u[:], b1_d[u])

                h1 = h1_p.tile([128, KO, C], mdt, tag="h1")

                # ---- mm1: h1[i, c] = gelu(sum_h W1[h,i] * X^T[h,c] + b1[i])
                for ic in range(N_W1C):
                    if ic not in w1cs:
                        w1cs[ic] = w1_p.tile([128, KO, W1CW], mdt, tag="w1c", name="w1c")
                        nc.sync.dma_start(w1cs[ic][:], cast(w1_d[u, ic]))
                    w1c = w1cs[ic]
                    for s in range(W1CW // 128):
                        i_out = ic * (W1CW // 128) + s
                        ps = ps1_p.tile([128, C], f32, tag="ps1")
                        for ko in range(KO):
                            nc.tensor.matmul(
                                ps[:, :CAP],
                                w1c[:, ko, s * 128:(s + 1) * 128],
                                xu[:, ko, :CAP],
                                start=(ko == 0),
                                stop=(ko == KO - 1),
                            )
                        nc.scalar.activation(
                            h1[:, i_out, :CAP], ps[:, :CAP], GELU,
                            bias=b1u[:, i_out:i_out + 1])

                # ---- mm2 (transposed): yT[h, c] = sum_i W2[i, h] * h1[i, c]
                # gating and b2 are applied on the host during scatter.
                oy = out_p.tile([128, KO, C], f32, tag="oy")
                for hcc in range(N_W2C):
                    w2c = w2_p.tile([128, KO, W2CW], mdt, tag="w2c")
                    nc.sync.dma_start(w2c[:], cast(w2_d[u, hcc]))
                    for s2 in range(W2CW // 128):
                        hk = hcc * (W2CW // 128) + s2
                        ps2 = ps2_p.tile([128, C], f32, tag="ps2")
                        for ko in range(KO):
                            nc.tensor.matmul(
                                ps2[:, :CAP],
                                w2c[:, ko, s2 * 128:(s2 + 1) * 128],
                                h1[:, ko, :CAP],
                                start=(ko == 0),
                                stop=(ko == KO - 1),
                            )
                        nc.vector.tensor_copy(oy[:, hk, :CAP], ps2[:, :CAP])
                        # drain finished output rows early so the final DMA
                        # (and the kernel tail) stays small
                        if hk % 2 == 1:
                            nc.sync.dma_start(
                                out_d[u, :, hk - 1:hk + 1, :CAP],
                                oy[:, hk - 1:hk + 1, :CAP])

    nc.compile()
    return nc


def _get_nc(wdt):
    if wdt not in _cache:
        _cache[wdt] = _build_nc(wdt)
    return _cache[wdt]


def _np_wdt(wdt):
    if wdt == "bf16":
        import ml_dtypes
        return np.dtype(ml_dtypes.bfloat16)
    if wdt == "fp16":
        return np.dtype(np.float16)
    return np.dtype(np.float32)


def _gelu_np(v):
    from scipy.special import erf
    v = v.astype(np.float32)
    return (0.5 * v * (1.0 + erf(v / np.sqrt(2.0)))).astype(np.float32)


def _tile_w1(w):
    # [H, I] -> [N_W1C, 128, KO, W1CW] with w1t[ic, p, ko, j] = w[ko*128+p, ic*W1CW+j]
    return w.reshape(KO, 128, N_W1C, W1CW).transpose(2, 1, 0, 3)


def _tile_w2(w):
    # [I, H] -> [N_W2C, 128, KO, W2CW]
    return w.reshape(KO, 128, N_W2C, W2CW).transpose(2, 1, 0, 3)


def _ensure_axon_hooks_stub():
    """bass_utils' axon trace path imports antenv.axon_hooks, which this
    image lacks; provide a no-op stub so a BASS_TRACE-enabled environment
    degrades gracefully instead of crashing."""
    import sys
    import types
    try:
        import antenv.axon_hooks  # noqa: F401
        return
    except ImportError:
        pass
    try:
        import antenv
    except ImportError:
        return
    mod = types.ModuleType("antenv.axon_hooks")
    holder = [None]
    mod.set_axon_ntff_profile_hook = lambda h: holder.__setitem__(0, h)
    mod.get_axon_ntff_profile_hook = lambda: holder[0]
    sys.modules["antenv.axon_hooks"] = mod
    antenv.axon_hooks = mod


def kernel(x, w1_shared, b1_shared, w2_shared, b2_shared,
           router_w, router_b, w1, b1, w2, b2):
    _ensure_axon_hooks_stub()
    from concourse.bass_utils import run_bass_kernel_spmd

    wdt = WORK_DTYPE
    ndt = _np_wdt(wdt)

    x = np.asarray(x, np.float32)
    w1 = np.asarray(w1, np.float32)
    b1 = np.asarray(b1, np.float32)
    w2 = np.asarray(w2, np.float32)
    b2 = np.asarray(b2, np.float32)
    w1_shared = np.asarray(w1_shared, np.float32)
    b1_shared = np.asarray(b1_shared, np.float32)
    w2_shared = np.asarray(w2_shared, np.float32)
    b2_shared = np.asarray(b2_shared, np.float32)
    router_w = np.asarray(router_w, np.float32)
    router_b = np.asarray(router_b, np.float32)

    xf = x.reshape(T, HID)

    # ---------------- host routing ----------------
    logits = xf @ router_w + router_b
    m = logits.max(-1, keepdims=True)
    ex = np.exp(logits - m, dtype=np.float32)
    affin = ex / ex.sum(-1, keepdims=True, dtype=np.float32)
    order = np.argsort(-affin, axis=-1, kind="stable")[:, :TOP_K]   # [T, K]
    vals = np.take_along_axis(affin, order, axis=-1)                # [T, K]

    # group (token, gate) pairs by expert
    flat_e = order.ravel()
    flat_t = np.repeat(np.arange(T), TOP_K)
    flat_g = vals.ravel()
    sort = np.argsort(flat_e, kind="stable")
    se, st, sg = flat_e[sort], flat_t[sort], flat_g[sort]
    starts = np.searchsorted(se, np.arange(E + 1))
    tok_by_e = [st[starts[e]:starts[e + 1]] for e in range(E)]
    gate_by_e = [sg[starts[e]:starts[e + 1]] for e in range(E)]

    # slot table: 64 expert slots; slot s = core*8 + unit.  Experts are
    # assigned by descending load rank: rank r -> core r%8, unit r//8, so
    # every core gets one expert from each load bucket and unit j's static
    # capacity CAPS[j] covers its bucket maximum.
    NSLOT = NCORES * 8
    slot_expert = [-1] * NSLOT
    slot_tok = [np.empty(0, np.int64)] * NSLOT
    slot_gate = [np.empty(0, np.float32)] * NSLOT
    ranked = sorted(range(E), key=lambda e: -len(tok_by_e[e]))
    overflow = []   # (expert, tokens, gates) beyond the primary slot cap
    for r, e in enumerate(ranked):
        s = (r % NCORES) * 8 + (r // NCORES)
        cap = CAPS[r // NCORES]
        slot_expert[s] = e
        slot_tok[s] = tok_by_e[e][:cap]
        slot_gate[s] = gate_by_e[e][:cap]
        if len(tok_by_e[e]) > cap:
            overflow.append((e, tok_by_e[e][cap:], gate_by_e[e][cap:]))
    # worst overflow spills into the spare slot 63 (unit 7, cap CAPS[7]);
    # anything further goes to an exact host fallback (rare).
    host_fallback = []
    if overflow:
        overflow.sort(key=lambda t: -len(t[1]))
        e0, t0, g0 = overflow[0]
        cap63 = CAPS[7]
        slot_expert[63] = e0
        slot_tok[63] = t0[:cap63]
        slot_gate[63] = g0[:cap63]
        if len(t0) > cap63:
            host_fallback.append((e0, t0[cap63:], g0[cap63:]))
        for e, t, g in overflow[1:]:
            host_fallback.append((e, t, g))

    # ---------------- build per-core device inputs ----------------
    # x transposed + partition-tiled: xT_t[ko, p, t] = x[t, ko*128+p]
    xT_t = np.ascontiguousarray(xf.T).astype(ndt).reshape(KO, 128, T)

    w1t_sh = _tile_w1(w1_shared[0]).astype(ndt)
    w2t_sh = _tile_w2(w2_shared[0]).astype(ndt)
    b1t_sh = b1_shared[0].reshape(KO, 128).T

    in_maps = []
    for c in range(NCORES):
        xg = np.zeros((UNITS, 128, KO, C), ndt)
        w1u = np.zeros((UNITS, N_W1C, 128, KO, W1CW), ndt)
        b1u = np.zeros((UNITS, 128, KO), np.float32)
        w2u = np.zeros((UNITS, N_W2C, 128, KO, W2CW), ndt)
        for u in range(8):
            s = c * 8 + u
            e = slot_expert[s]
            if e < 0 or len(slot_tok[s]) == 0:
                continue
            n = len(slot_tok[s])
            idx = np.zeros(C, np.int64)
            idx[:n] = slot_tok[s]
            xg[u] = xT_t[:, :, idx].swapaxes(0, 1)
            w1u[u] = _tile_w1(w1[e]).astype(ndt)
            b1u[u] = b1[e].reshape(KO, 128).T
            w2u[u] = _tile_w2(w2[e]).astype(ndt)
        # shared-expert unit
        xg[8] = xT_t[:, :, c * TSH:(c + 1) * TSH].swapaxes(0, 1)
        w1u[8] = w1t_sh
        b1u[8] = b1t_sh
        w2u[8] = w2t_sh
        in_maps.append({"xg": xg, "w1": w1u, "b1": b1u, "w2": w2u})

    # ---------------- run on 8 cores ----------------
    nc = _get_nc(wdt)
    res = run_bass_kernel_spmd(nc, in_maps, core_ids=list(range(NCORES)))
    outs = [r["out"] for r in res.results]   # [UNITS, 128, CM, HID] each

    # ---------------- host unshard / scatter ----------------
    # device output is transposed: outs[c][u][p, hk, c'] = y[c', hk*128+p]
    def untile_y(o, n):
        return o.transpose(1, 0, 2).reshape(HID, C)[:, :n].T

    acc = np.zeros((T, HID), np.float32)     # shared + routed
    # shared expert (unit 8 on each core), gate 1, + b2_shared
    for c in range(NCORES):
        ys = untile_y(outs[c][8], TSH)
        acc[c * TSH:(c + 1) * TSH] = ys + b2_shared[0]
    # routed experts: gate * (y + b2), scattered by token
    for s in range(NCORES * 8):
        e = slot_expert[s]
        n = len(slot_tok[s])
        if e < 0 or n == 0:
            continue
        ye = untile_y(outs[s // 8][s % 8], n)
        # token indices are unique within one slot, so fancy += is safe
        acc[slot_tok[s]] += slot_gate[s][:, None] * (ye + b2[e][None, :])
    # exact host fallback for overflow beyond device capacity
    for e, toks, gs in host_fallback:
        h = _gelu_np(xf[toks] @ w1[e] + b1[e])
        acc[toks] += gs[:, None] * (h @ w2[e] + b2[e])

    return (acc + xf).reshape(B, S, HID).astype(np.float32)



# revision 9
# speedup vs baseline: 1.5300x; 1.5300x over previous
"""MoE (63 routed experts, top-7, 1 shared expert) Trainium2 Bass kernel.

Strategy (expert parallelism, per sharding hint):
  - Host: router matmul + softmax + top-k (tiny: 0.7 GFLOP vs 220 GFLOP of
    expert FFNs), token gather per expert.
  - Device (8 NeuronCores, SPMD): each core runs 9 "units" of identical
    shape: 8 routed-expert slots (64 slots globally = 63 experts + 1
    overflow slot) and 1 shared-expert slot over a 1/8 token slice.
    Each unit: h = gelu(XeT^T @ W1 + b1); y = gate * (h @ W2), with
    full-rate matmuls (float32r or bf16), GELU fused into the PSUM
    eviction on the scalar engine, gating fused into the PSUM eviction on
    the vector engine.  Weights are host-pretiled into chunk-contiguous
    layout so every DMA is a flat [128 x bytes] block.
  - Host: scatter-add gated expert outputs (+ gate*b2), add shared out,
    bias and residual.

Experts are assigned to slots by descending load rank with static per-unit
token capacities (CAPS); both matmul layers' free dim is the capacity, so
PE cost tracks actual expert load.  Overload spills into the spare 64th
slot and, beyond that, to an exact host-side FFN for the few excess
tokens.  Gating and b2 are applied on the host during the scatter.
"""

import os

import numpy as np

B, S, HID = 2, 2048, 1280
E = 63
I = 1280
TOP_K = 7
NCORES = 8
UNITS = 9          # 8 expert slots + 1 shared-expert slot
C = 512            # token capacity per expert slot
CM = C // 128      # 4
KO = HID // 128    # 10 contraction chunks
T = B * S          # 4096
TSH = T // NCORES  # 512 shared-expert tokens per core

W1CW = 256          # w1 chunk width along I (2 lhsT column groups)
W2CW = 256          # w2 chunk width along H (2 lhsT column groups)
N_W1C = I // W1CW   # 5
N_W2C = HID // W2CW  # 5

# Per-unit-index token capacities. Experts are assigned to slots by load
# rank (rank r -> core r%8, unit r//8), so unit j only ever sees the j-th
# bucket of the descending load distribution; caps cover the bucket maxima
# of any near-uniform routing with margin. Uncovered overflow goes to the
# spare slot 63 and, beyond that, to an exact host fallback.
CAPS = [512, 500, 484, 472, 460, 448, 440, 420, C]   # unit 8 = shared

# "f32r": fp32 data, full-rate float32r matmuls (most accurate).
# "bf16": bf16 weights+activations, fp32 accumulate (halves DMA traffic).
# "fp16": like bf16 but 4x finer mantissa; all values here are well within
#         fp16 range, so this is strictly more accurate at the same speed.
# "fp8":  e4m3 weights+activations with DoubleRow matmuls (0.5 cycles/row,
#         ~2x PE throughput).  Inputs are pre-scaled into e4m3's sweet spot
#         (S_X for x, S_W for both weight matrices); the mm1 descale is
#         folded into the GELU's input scale, the mm2 descale into the host
#         scatter.  fp16 device output halves the drain DMA.
WORK_DTYPE = os.environ.get("MOE_WDT", "fp8")

S_X = 16.0          # x -> fp8 scale
S_W = 64.0          # w1, w2 -> fp8 scale
INV_S1 = 1.0 / (S_X * S_W)   # PSUM descale before GELU (mm1)
INV_S2 = 1.0 / S_W           # host descale of mm2 output

_cache = {}


def _build_nc(wdt):
    import concourse.mybir as mybir
    import concourse.tile as tile
    from concourse import bacc

    f32 = mybir.dt.float32
    GELU = mybir.ActivationFunctionType.Gelu
    if os.environ.get("MOE_SIM_NOGELU"):      # CoreSim lacks Gelu; layout-
        GELU = mybir.ActivationFunctionType.Identity   # check runs use this
    fp8 = wdt == "fp8"
    DR = mybir.MatmulPerfMode.DoubleRow if fp8 else None
    if wdt == "f32r":
        mdt = mybir.dt.float32r
        ddt = f32    # dram dtype for weight/activation tensors
        odt = f32
        bufs = dict(xu=2, h1=2, w1c=3, w2c=3, ou=2)
    elif fp8:
        mdt = mybir.dt.float8e4
        ddt = mdt
        odt = mybir.dt.float16
        bufs = dict(xu=3, h1=3, w1c=4, w2c=4, ou=2)
    else:
        mdt = mybir.dt.float16 if wdt == "fp16" else mybir.dt.bfloat16
        ddt = mdt
        odt = f32
        bufs = dict(xu=3, h1=3, w1c=4, w2c=4, ou=2)

    nc = bacc.Bacc(None, target_bir_lowering=False)

    xg_d = nc.dram_tensor("xg", [UNITS, 128, KO, C], ddt, kind="ExternalInput")
    w1_d = nc.dram_tensor("w1", [UNITS, N_W1C, 128, KO, W1CW], ddt,
                          kind="ExternalInput")
    b1_d = nc.dram_tensor("b1", [UNITS, 128, KO], f32, kind="ExternalInput")
    w2_d = nc.dram_tensor("w2", [UNITS, N_W2C, 128, KO, W2CW], ddt,
                          kind="ExternalInput")
    # transposed output: out[u, p, hk, c] = y[token c, h = hk*128+p]
    out_d = nc.dram_tensor("out", [UNITS, 128, KO, C], odt, kind="ExternalOutput")

    def cast(ap):
        return ap.bitcast(mdt) if wdt == "f32r" else ap

    with tile.TileContext(nc) as tc:
        with tc.tile_pool(name="xg_p", bufs=bufs["xu"]) as xg_p, \
             tc.tile_pool(name="h1_p", bufs=bufs["h1"]) as h1_p, \
             tc.tile_pool(name="w1_p", bufs=bufs["w1c"]) as w1_p, \
             tc.tile_pool(name="w2_p", bufs=bufs["w2c"]) as w2_p, \
             tc.tile_pool(name="out_p", bufs=bufs["ou"]) as out_p, \
             tc.tile_pool(name="sm_p", bufs=3) as sm_p, \
             tc.tile_pool(name="ps1_p", bufs=3, space="PSUM") as ps1_p, \
             tc.tile_pool(name="ps2_p", bufs=4, space="PSUM") as ps2_p:

            for u in range(UNITS):
                CAP = CAPS[u]
                w1cs = {}
                # first w1 chunk ahead of everything else the unit needs
                w1cs[0] = w1_p.tile([128, KO, W1CW], mdt, tag="w1c", name="w1c")
                nc.sync.dma_start(w1cs[0][:], cast(w1_d[u, 0]))
                xu = xg_p.tile([128, KO, C], mdt, tag="xu")
                # split halves so the first matmuls can start sooner
                nc.sync.dma_start(xu[:, :KO // 2, :CAP],
                                  cast(xg_d[u, :, :KO // 2, :CAP]))
                nc.sync.dma_start(xu[:, KO // 2:, :CAP],
                                  cast(xg_d[u, :, KO // 2:, :CAP]))
                b1u = sm_p.tile([128, KO], f32, tag="b1u")
                nc.sync.dma_start(b1u[:], b1_d[u])

                h1 = h1_p.tile([128, KO, C], mdt, tag="h1")

                # ---- mm1: h1[i, c] = gelu(sum_h W1[h,i] * X^T[h,c] + b1[i])
                for ic in range(N_W1C):
                    if ic not in w1cs:
                        w1cs[ic] = w1_p.tile([128, KO, W1CW], mdt, tag="w1c", name="w1c")
                        nc.sync.dma_start(w1cs[ic][:], cast(w1_d[u, ic]))
                    w1c = w1cs[ic]
                    for s in range(W1CW // 128):
                        i_out = ic * (W1CW // 128) + s
                        ps = ps1_p.tile([128, C], f32, tag="ps1")
                        if fp8:
                            for k2 in range(KO // 2):
                                nc.tensor.matmul(
                                    ps[:, :CAP],
                                    w1c[:, 2 * k2:2 * k2 + 2, s * 128:(s + 1) * 128],
                                    xu[:, 2 * k2:2 * k2 + 2, :CAP],
                                    start=(k2 == 0),
                                    stop=(k2 == KO // 2 - 1),
                                    perf_mode=DR,
                                )
                        else:
                            for ko in range(KO):
                                nc.tensor.matmul(
                                    ps[:, :CAP],
                                    w1c[:, ko, s * 128:(s + 1) * 128],
                                    xu[:, ko, :CAP],
                                    start=(ko == 0),
                                    stop=(ko == KO - 1),
                                )
                        nc.scalar.activation(
                            h1[:, i_out, :CAP], ps[:, :CAP], GELU,
                            bias=b1u[:, i_out:i_out + 1],
                            scale=INV_S1 if fp8 else 1.0)

                # ---- mm2 (transposed): yT[h, c] = sum_i W2[i, h] * h1[i, c]
                # gating and b2 are applied on the host during scatter.
                oy = out_p.tile([128, KO, C], odt, tag="oy")
                for hcc in range(N_W2C):
                    w2c = w2_p.tile([128, KO, W2CW], mdt, tag="w2c")
                    nc.sync.dma_start(w2c[:], cast(w2_d[u, hcc]))
                    for s2 in range(W2CW // 128):
                        hk = hcc * (W2CW // 128) + s2
                        ps2 = ps2_p.tile([128, C], f32, tag="ps2")
                        if fp8:
                            for k2 in range(KO // 2):
                                nc.tensor.matmul(
                                    ps2[:, :CAP],
                                    w2c[:, 2 * k2:2 * k2 + 2, s2 * 128:(s2 + 1) * 128],
                                    h1[:, 2 * k2:2 * k2 + 2, :CAP],
                                    start=(k2 == 0),
                                    stop=(k2 == KO // 2 - 1),
                                    perf_mode=DR,
                                )
                        else:
                            for ko in range(KO):
                                nc.tensor.matmul(
                                    ps2[:, :CAP],
                                    w2c[:, ko, s2 * 128:(s2 + 1) * 128],
                                    h1[:, ko, :CAP],
                                    start=(ko == 0),
                                    stop=(ko == KO - 1),
                                )
                        nc.vector.tensor_copy(oy[:, hk, :CAP], ps2[:, :CAP])
                        # drain finished output rows early so the final DMA
                        # (and the kernel tail) stays small
                        if hk % 2 == 1:
                            nc.sync.dma_start(
                                out_d[u, :, hk - 1:hk + 1, :CAP],
                                oy[:, hk - 1:hk + 1, :CAP])

    nc.compile()
    return nc


def _get_nc(wdt):
    if wdt not in _cache:
        _cache[wdt] = _build_nc(wdt)
    return _cache[wdt]


def _np_wdt(wdt):
    if wdt == "bf16":
        import ml_dtypes
        return np.dtype(ml_dtypes.bfloat16)
    if wdt == "fp16":
        return np.dtype(np.float16)
    if wdt == "fp8":
        import ml_dtypes
        # TRN FP8_EXP4: bias 7, max normal 240 — ml_dtypes' IEEE e4m3
        return np.dtype(ml_dtypes.float8_e4m3)
    return np.dtype(np.float32)


def _gelu_np(v):
    from scipy.special import erf
    v = v.astype(np.float32)
    return (0.5 * v * (1.0 + erf(v / np.sqrt(2.0)))).astype(np.float32)


def _tile_w1(w):
    # [H, I] -> [N_W1C, 128, KO, W1CW] with w1t[ic, p, ko, j] = w[ko*128+p, ic*W1CW+j]
    return w.reshape(KO, 128, N_W1C, W1CW).transpose(2, 1, 0, 3)


def _tile_w2(w):
    # [I, H] -> [N_W2C, 128, KO, W2CW]
    return w.reshape(KO, 128, N_W2C, W2CW).transpose(2, 1, 0, 3)


def _ensure_axon_hooks_stub():
    """bass_utils' axon trace path imports antenv.axon_hooks, which this
    image lacks; provide a no-op stub so a BASS_TRACE-enabled environment
    degrades gracefully instead of crashing."""
    import sys
    import types
    try:
        import antenv.axon_hooks  # noqa: F401
        return
    except ImportError:
        pass
    try:
        import antenv
    except ImportError:
        return
    mod = types.ModuleType("antenv.axon_hooks")
    holder = [None]
    mod.set_axon_ntff_profile_hook = lambda h: holder.__setitem__(0, h)
    mod.get_axon_ntff_profile_hook = lambda: holder[0]
    sys.modules["antenv.axon_hooks"] = mod
    antenv.axon_hooks = mod


def kernel(x, w1_shared, b1_shared, w2_shared, b2_shared,
           router_w, router_b, w1, b1, w2, b2):
    _ensure_axon_hooks_stub()
    from concourse.bass_utils import run_bass_kernel_spmd

    wdt = WORK_DTYPE
    ndt = _np_wdt(wdt)

    x = np.asarray(x, np.float32)
    w1 = np.asarray(w1, np.float32)
    b1 = np.asarray(b1, np.float32)
    w2 = np.asarray(w2, np.float32)
    b2 = np.asarray(b2, np.float32)
    w1_shared = np.asarray(w1_shared, np.float32)
    b1_shared = np.asarray(b1_shared, np.float32)
    w2_shared = np.asarray(w2_shared, np.float32)
    b2_shared = np.asarray(b2_shared, np.float32)
    router_w = np.asarray(router_w, np.float32)
    router_b = np.asarray(router_b, np.float32)

    xf = x.reshape(T, HID)

    # ---------------- host routing ----------------
    logits = xf @ router_w + router_b
    m = logits.max(-1, keepdims=True)
    ex = np.exp(logits - m, dtype=np.float32)
    affin = ex / ex.sum(-1, keepdims=True, dtype=np.float32)
    order = np.argsort(-affin, axis=-1, kind="stable")[:, :TOP_K]   # [T, K]
    vals = np.take_along_axis(affin, order, axis=-1)                # [T, K]

    # group (token, gate) pairs by expert
    flat_e = order.ravel()
    flat_t = np.repeat(np.arange(T), TOP_K)
    flat_g = vals.ravel()
    sort = np.argsort(flat_e, kind="stable")
    se, st, sg = flat_e[sort], flat_t[sort], flat_g[sort]
    starts = np.searchsorted(se, np.arange(E + 1))
    tok_by_e = [st[starts[e]:starts[e + 1]] for e in range(E)]
    gate_by_e = [sg[starts[e]:starts[e + 1]] for e in range(E)]

    # slot table: 64 expert slots; slot s = core*8 + unit.  Experts are
    # assigned by descending load rank: rank r -> core r%8, unit r//8, so
    # every core gets one expert from each load bucket and unit j's static
    # capacity CAPS[j] covers its bucket maximum.
    NSLOT = NCORES * 8
    slot_expert = [-1] * NSLOT
    slot_tok = [np.empty(0, np.int64)] * NSLOT
    slot_gate = [np.empty(0, np.float32)] * NSLOT
    ranked = sorted(range(E), key=lambda e: -len(tok_by_e[e]))
    overflow = []   # (expert, tokens, gates) beyond the primary slot cap
    for r, e in enumerate(ranked):
        s = (r % NCORES) * 8 + (r // NCORES)
        cap = CAPS[r // NCORES]
        slot_expert[s] = e
        slot_tok[s] = tok_by_e[e][:cap]
        slot_gate[s] = gate_by_e[e][:cap]
        if len(tok_by_e[e]) > cap:
            overflow.append((e, tok_by_e[e][cap:], gate_by_e[e][cap:]))
    # worst overflow spills into the spare slot 63 (unit 7, cap CAPS[7]);
    # anything further goes to an exact host fallback (rare).
    host_fallback = []
    if overflow:
        overflow.sort(key=lambda t: -len(t[1]))
        e0, t0, g0 = overflow[0]
        cap63 = CAPS[7]
        slot_expert[63] = e0
        slot_tok[63] = t0[:cap63]
        slot_gate[63] = g0[:cap63]
        if len(t0) > cap63:
            host_fallback.append((e0, t0[cap63:], g0[cap63:]))
        for e, t, g in overflow[1:]:
            host_fallback.append((e, t, g))

    # ---------------- build per-core device inputs ----------------
    fp8 = wdt == "fp8"
    sx = S_X if fp8 else 1.0
    sw = S_W if fp8 else 1.0

    def q(a):
        # clip keeps accidental outliers out of fp8-inf territory (TRN
        # e4m3 max normal is 240); no-op for 16/32-bit working dtypes
        return (np.clip(a, -240.0, 240.0) if fp8 else a).astype(ndt)

    # x transposed + partition-tiled: xT_t[ko, p, t] = x[t, ko*128+p]
    xT_t = q(np.ascontiguousarray(xf.T) * sx).reshape(KO, 128, T)

    w1t_sh = q(_tile_w1(w1_shared[0]) * sw)
    w2t_sh = q(_tile_w2(w2_shared[0]) * sw)
    b1t_sh = b1_shared[0].reshape(KO, 128).T

    in_maps = []
    for c in range(NCORES):
        xg = np.zeros((UNITS, 128, KO, C), ndt)
        w1u = np.zeros((UNITS, N_W1C, 128, KO, W1CW), ndt)
        b1u = np.zeros((UNITS, 128, KO), np.float32)
        w2u = np.zeros((UNITS, N_W2C, 128, KO, W2CW), ndt)
        for u in range(8):
            s = c * 8 + u
            e = slot_expert[s]
            if e < 0 or len(slot_tok[s]) == 0:
                continue
            n = len(slot_tok[s])
            idx = np.zeros(C, np.int64)
            idx[:n] = slot_tok[s]
            xg[u] = xT_t[:, :, idx].swapaxes(0, 1)
            w1u[u] = q(_tile_w1(w1[e]) * sw)
            b1u[u] = b1[e].reshape(KO, 128).T
            w2u[u] = q(_tile_w2(w2[e]) * sw)
        # shared-expert unit
        xg[8] = xT_t[:, :, c * TSH:(c + 1) * TSH].swapaxes(0, 1)
        w1u[8] = w1t_sh
        b1u[8] = b1t_sh
        w2u[8] = w2t_sh
        in_maps.append({"xg": xg, "w1": w1u, "b1": b1u, "w2": w2u})

    # ---------------- run on 8 cores ----------------
    nc = _get_nc(wdt)
    res = run_bass_kernel_spmd(nc, in_maps, core_ids=list(range(NCORES)))
    outs = [r["out"] for r in res.results]   # [UNITS, 128, CM, HID] each

    # ---------------- host unshard / scatter ----------------
    # device output is transposed: outs[c][u][p, hk, c'] = y[c', hk*128+p]
    inv2 = INV_S2 if fp8 else 1.0   # undo the S_W scale baked into mm2

    def untile_y(o, n):
        y = o.transpose(1, 0, 2).reshape(HID, C)[:, :n].T.astype(np.float32)
        return y * inv2 if fp8 else y

    acc = np.zeros((T, HID), np.float32)     # shared + routed
    # shared expert (unit 8 on each core), gate 1, + b2_shared
    for c in range(NCORES):
        ys = untile_y(outs[c][8], TSH)
        acc[c * TSH:(c + 1) * TSH] = ys + b2_shared[0]
    # routed experts: gate * (y + b2), scattered by token
    for s in range(NCORES * 8):
        e = slot_expert[s]
        n = len(slot_tok[s])
        if e < 0 or n == 0:
            continue
        ye = untile_y(outs[s // 8][s % 8], n)
        # token indices are unique within one slot, so fancy += is safe
        acc[slot_tok[s]] += slot_gate[s][:, None] * (ye + b2[e][None, :])
    # exact host fallback for overflow beyond device capacity
    for e, toks, gs in host_fallback:
        h = _gelu_np(xf[toks] @ w1[e] + b1[e])
        acc[toks] += gs[:, None] * (h @ w2[e] + b2[e])

    return (acc + xf).reshape(B, S, HID).astype(np.float32)



# revision 12
# speedup vs baseline: 1.8259x; 1.1934x over previous
"""MoE (63 routed experts, top-7, 1 shared expert) Trainium2 Bass kernel.

Strategy (expert parallelism, per sharding hint):
  - Host: router matmul + softmax + top-k (tiny: 0.7 GFLOP vs 220 GFLOP of
    expert FFNs), token gather per expert.
  - Device (8 NeuronCores, SPMD): each core runs 9 "units" of identical
    shape: 8 routed-expert slots (64 slots globally = 63 experts + 1
    overflow slot) and 1 shared-expert slot over a 1/8 token slice.
    Each unit: h = gelu(XeT^T @ W1 + b1); y = gate * (h @ W2), with
    full-rate matmuls (float32r or bf16), GELU fused into the PSUM
    eviction on the scalar engine, gating fused into the PSUM eviction on
    the vector engine.  Weights are host-pretiled into chunk-contiguous
    layout so every DMA is a flat [128 x bytes] block.
  - Host: scatter-add gated expert outputs (+ gate*b2), add shared out,
    bias and residual.

Experts are assigned to slots by descending load rank with static per-unit
token capacities (CAPS); both matmul layers' free dim is the capacity, so
PE cost tracks actual expert load.  Overload spills into the spare 64th
slot and, beyond that, to an exact host-side FFN for the few excess
tokens.  Gating and b2 are applied on the host during the scatter.
"""

import os

import numpy as np

B, S, HID = 2, 2048, 1280
E = 63
I = 1280
TOP_K = 7
NCORES = 8
UNITS = 9          # 8 expert slots + 1 shared-expert slot
C = 512            # token capacity per expert slot
CM = C // 128      # 4
KO = HID // 128    # 10 contraction chunks
T = B * S          # 4096
TSH = T // NCORES  # 512 shared-expert tokens per core

W1CW = 256          # w1 chunk width along I (2 lhsT column groups)
W2CW = 256          # w2 chunk width along H (2 lhsT column groups)
N_W1C = I // W1CW   # 5
N_W2C = HID // W2CW  # 5

# Per-unit-index token capacities. Experts are assigned to slots by load
# rank (rank r -> core r%8, unit r//8), so unit j only ever sees the j-th
# bucket of the descending load distribution; caps cover the bucket maxima
# of any near-uniform routing with margin. Uncovered overflow goes to the
# spare slot 63 and, beyond that, to an exact host fallback.
CAPS = [512, 500, 484, 472, 460, 448, 440, 420, C]   # unit 8 = shared

# "f32r": fp32 data, full-rate float32r matmuls (most accurate).
# "bf16": bf16 weights+activations, fp32 accumulate (halves DMA traffic).
# "fp16": like bf16 but 4x finer mantissa; all values here are well within
#         fp16 range, so this is strictly more accurate at the same speed.
# "fp8":  e4m3 weights+activations with DoubleRow matmuls (0.5 cycles/row,
#         ~2x PE throughput).  Inputs are pre-scaled into e4m3's sweet spot
#         (S_X for x, S_W for both weight matrices); the mm1 descale is
#         folded into the GELU's input scale, the mm2 descale into the host
#         scatter.  fp16 device output halves the drain DMA.
WORK_DTYPE = os.environ.get("MOE_WDT", "fp8")

S_X = 16.0          # x -> fp8 scale
S_W = 64.0          # w1, w2 -> fp8 scale
INV_S1 = 1.0 / (S_X * S_W)   # PSUM descale before GELU (mm1)
INV_S2 = 1.0 / S_W           # host descale of mm2 output

_cache = {}


def _build_nc(wdt):
    import concourse.mybir as mybir
    import concourse.tile as tile
    from concourse import bacc

    f32 = mybir.dt.float32
    GELU = mybir.ActivationFunctionType.Gelu
    if os.environ.get("MOE_SIM_NOGELU"):      # CoreSim lacks Gelu; layout-
        GELU = mybir.ActivationFunctionType.Identity   # check runs use this
    fp8 = wdt == "fp8"
    DR = mybir.MatmulPerfMode.DoubleRow if fp8 else None
    if wdt == "f32r":
        mdt = mybir.dt.float32r
        ddt = f32    # dram dtype for weight/activation tensors
        odt = f32
        bufs = dict(xu=2, h1=2, w1c=3, w2c=3, ou=2)
    elif fp8:
        mdt = mybir.dt.float8e4
        ddt = mdt
        odt = mybir.dt.float16
        bufs = dict(xu=4, h1=3, w1c=6, w2c=4, ou=3)
    else:
        mdt = mybir.dt.float16 if wdt == "fp16" else mybir.dt.bfloat16
        ddt = mdt
        odt = f32
        bufs = dict(xu=3, h1=3, w1c=4, w2c=4, ou=2)

    nc = bacc.Bacc(None, target_bir_lowering=False)

    xg_d = nc.dram_tensor("xg", [UNITS, 128, KO, C], ddt, kind="ExternalInput")
    w1_d = nc.dram_tensor("w1", [UNITS, N_W1C, 128, KO, W1CW], ddt,
                          kind="ExternalInput")
    b1_d = nc.dram_tensor("b1", [UNITS, 128, KO], f32, kind="ExternalInput")
    w2_d = nc.dram_tensor("w2", [UNITS, N_W2C, 128, KO, W2CW], ddt,
                          kind="ExternalInput")
    # transposed output: out[u, p, hk, c] = y[token c, h = hk*128+p]
    out_d = nc.dram_tensor("out", [UNITS, 128, KO, C], odt, kind="ExternalOutput")

    def cast(ap):
        return ap.bitcast(mdt) if wdt == "f32r" else ap

    with tile.TileContext(nc) as tc:
        with tc.tile_pool(name="xg_p", bufs=bufs["xu"]) as xg_p, \
             tc.tile_pool(name="h1_p", bufs=bufs["h1"]) as h1_p, \
             tc.tile_pool(name="w1_p", bufs=bufs["w1c"]) as w1_p, \
             tc.tile_pool(name="w2_p", bufs=bufs["w2c"]) as w2_p, \
             tc.tile_pool(name="out_p", bufs=bufs["ou"]) as out_p, \
             tc.tile_pool(name="sm_p", bufs=3) as sm_p, \
             tc.tile_pool(name="ps1_p", bufs=3, space="PSUM") as ps1_p, \
             tc.tile_pool(name="ps2_p", bufs=4, space="PSUM") as ps2_p:

            for u in range(UNITS):
                CAP = CAPS[u]
                w1cs = {}
                # first w1 chunk ahead of everything else the unit needs
                w1cs[0] = w1_p.tile([128, KO, W1CW], mdt, tag="w1c", name="w1c")
                xu = xg_p.tile([128, KO, C], mdt, tag="xu")
                if u == 0:
                    # head-latency trim: land just the slices the first PSUM
                    # chain needs, then the rest
                    nc.sync.dma_start(w1cs[0][:, :2], cast(w1_d[u, 0, :, :2]))
                    nc.sync.dma_start(xu[:, :2, :CAP],
                                      cast(xg_d[u, :, :2, :CAP]))
                    nc.sync.dma_start(w1cs[0][:, 2:], cast(w1_d[u, 0, :, 2:]))
                    nc.sync.dma_start(xu[:, 2:, :CAP],
                                      cast(xg_d[u, :, 2:, :CAP]))
                else:
                    nc.sync.dma_start(w1cs[0][:], cast(w1_d[u, 0]))
                    # split halves so the first matmuls can start sooner
                    nc.sync.dma_start(xu[:, :KO // 2, :CAP],
                                      cast(xg_d[u, :, :KO // 2, :CAP]))
                    nc.sync.dma_start(xu[:, KO // 2:, :CAP],
                                      cast(xg_d[u, :, KO // 2:, :CAP]))
                b1u = sm_p.tile([128, KO], f32, tag="b1u")
                nc.sync.dma_start(b1u[:], b1_d[u])

                h1 = h1_p.tile([128, KO, C], mdt, tag="h1")

                # ---- mm1: h1[i, c] = gelu(sum_h W1[h,i] * X^T[h,c] + b1[i])
                for ic in range(N_W1C):
                    if ic not in w1cs:
                        w1cs[ic] = w1_p.tile([128, KO, W1CW], mdt, tag="w1c", name="w1c")
                        nc.sync.dma_start(w1cs[ic][:], cast(w1_d[u, ic]))
                    w1c = w1cs[ic]
                    for s in range(W1CW // 128):
                        i_out = ic * (W1CW // 128) + s
                        ps = ps1_p.tile([128, C], f32, tag="ps1")
                        if fp8:
                            for k2 in range(KO // 2):
                                nc.tensor.matmul(
                                    ps[:, :CAP],
                                    w1c[:, 2 * k2:2 * k2 + 2, s * 128:(s + 1) * 128],
                                    xu[:, 2 * k2:2 * k2 + 2, :CAP],
                                    start=(k2 == 0),
                                    stop=(k2 == KO // 2 - 1),
                                    perf_mode=DR,
                                )
                        else:
                            for ko in range(KO):
                                nc.tensor.matmul(
                                    ps[:, :CAP],
                                    w1c[:, ko, s * 128:(s + 1) * 128],
                                    xu[:, ko, :CAP],
                                    start=(ko == 0),
                                    stop=(ko == KO - 1),
                                )
                        nc.scalar.activation(
                            h1[:, i_out, :CAP], ps[:, :CAP], GELU,
                            bias=b1u[:, i_out:i_out + 1],
                            scale=INV_S1 if fp8 else 1.0)

                # ---- mm2 (transposed): yT[h, c] = sum_i W2[i, h] * h1[i, c]
                # gating and b2 are applied on the host during scatter.
                oy = out_p.tile([128, KO, C], odt, tag="oy")
                for hcc in range(N_W2C):
                    w2c = w2_p.tile([128, KO, W2CW], mdt, tag="w2c")
                    nc.sync.dma_start(w2c[:], cast(w2_d[u, hcc]))
                    for s2 in range(W2CW // 128):
                        hk = hcc * (W2CW // 128) + s2
                        ps2 = ps2_p.tile([128, C], f32, tag="ps2")
                        if fp8:
                            for k2 in range(KO // 2):
                                nc.tensor.matmul(
                                    ps2[:, :CAP],
                                    w2c[:, 2 * k2:2 * k2 + 2, s2 * 128:(s2 + 1) * 128],
                                    h1[:, 2 * k2:2 * k2 + 2, :CAP],
                                    start=(k2 == 0),
                                    stop=(k2 == KO // 2 - 1),
                                    perf_mode=DR,
                                )
                        else:
                            for ko in range(KO):
                                nc.tensor.matmul(
                                    ps2[:, :CAP],
                                    w2c[:, ko, s2 * 128:(s2 + 1) * 128],
                                    h1[:, ko, :CAP],
                                    start=(ko == 0),
                                    stop=(ko == KO - 1),
                                )
                        nc.vector.tensor_copy(oy[:, hk, :CAP], ps2[:, :CAP])
                        # drain finished output rows early so the final DMA
                        # (and the kernel tail) stays small; a separate DMA
                        # queue (gpsimd) keeps drains from head-of-line
                        # blocking the next unit's input loads on sync
                        if hk % 2 == 1:
                            nc.gpsimd.dma_start(
                                out_d[u, :, hk - 1:hk + 1, :CAP],
                                oy[:, hk - 1:hk + 1, :CAP])

    nc.compile()
    return nc


def _get_nc(wdt):
    if wdt not in _cache:
        _cache[wdt] = _build_nc(wdt)
    return _cache[wdt]


def _np_wdt(wdt):
    if wdt == "bf16":
        import ml_dtypes
        return np.dtype(ml_dtypes.bfloat16)
    if wdt == "fp16":
        return np.dtype(np.float16)
    if wdt == "fp8":
        import ml_dtypes
        # TRN FP8_EXP4: bias 7, max normal 240 — ml_dtypes' IEEE e4m3
        return np.dtype(ml_dtypes.float8_e4m3)
    return np.dtype(np.float32)


def _gelu_np(v):
    from scipy.special import erf
    v = v.astype(np.float32)
    return (0.5 * v * (1.0 + erf(v / np.sqrt(2.0)))).astype(np.float32)


def _tile_w1(w):
    # [H, I] -> [N_W1C, 128, KO, W1CW] with w1t[ic, p, ko, j] = w[ko*128+p, ic*W1CW+j]
    return w.reshape(KO, 128, N_W1C, W1CW).transpose(2, 1, 0, 3)


def _tile_w2(w):
    # [I, H] -> [N_W2C, 128, KO, W2CW]
    return w.reshape(KO, 128, N_W2C, W2CW).transpose(2, 1, 0, 3)


def _ensure_axon_hooks_stub():
    """bass_utils' axon trace path imports antenv.axon_hooks, which this
    image lacks; provide a no-op stub so a BASS_TRACE-enabled environment
    degrades gracefully instead of crashing."""
    import sys
    import types
    try:
        import antenv.axon_hooks  # noqa: F401
        return
    except ImportError:
        pass
    try:
        import antenv
    except ImportError:
        return
    mod = types.ModuleType("antenv.axon_hooks")
    holder = [None]
    mod.set_axon_ntff_profile_hook = lambda h: holder.__setitem__(0, h)
    mod.get_axon_ntff_profile_hook = lambda: holder[0]
    sys.modules["antenv.axon_hooks"] = mod
    antenv.axon_hooks = mod


def kernel(x, w1_shared, b1_shared, w2_shared, b2_shared,
           router_w, router_b, w1, b1, w2, b2):
    _ensure_axon_hooks_stub()
    from concourse.bass_utils import run_bass_kernel_spmd

    wdt = WORK_DTYPE
    ndt = _np_wdt(wdt)

    x = np.asarray(x, np.float32)
    w1 = np.asarray(w1, np.float32)
    b1 = np.asarray(b1, np.float32)
    w2 = np.asarray(w2, np.float32)
    b2 = np.asarray(b2, np.float32)
    w1_shared = np.asarray(w1_shared, np.float32)
    b1_shared = np.asarray(b1_shared, np.float32)
    w2_shared = np.asarray(w2_shared, np.float32)
    b2_shared = np.asarray(b2_shared, np.float32)
    router_w = np.asarray(router_w, np.float32)
    router_b = np.asarray(router_b, np.float32)

    xf = x.reshape(T, HID)

    # ---------------- host routing ----------------
    logits = xf @ router_w + router_b
    m = logits.max(-1, keepdims=True)
    ex = np.exp(logits - m, dtype=np.float32)
    affin = ex / ex.sum(-1, keepdims=True, dtype=np.float32)
    order = np.argsort(-affin, axis=-1, kind="stable")[:, :TOP_K]   # [T, K]
    vals = np.take_along_axis(affin, order, axis=-1)                # [T, K]

    # group (token, gate) pairs by expert
    flat_e = order.ravel()
    flat_t = np.repeat(np.arange(T), TOP_K)
    flat_g = vals.ravel()
    sort = np.argsort(flat_e, kind="stable")
    se, st, sg = flat_e[sort], flat_t[sort], flat_g[sort]
    starts = np.searchsorted(se, np.arange(E + 1))
    tok_by_e = [st[starts[e]:starts[e + 1]] for e in range(E)]
    gate_by_e = [sg[starts[e]:starts[e + 1]] for e in range(E)]

    # slot table: 64 expert slots; slot s = core*8 + unit.  Experts are
    # assigned by descending load rank: rank r -> core r%8, unit r//8, so
    # every core gets one expert from each load bucket and unit j's static
    # capacity CAPS[j] covers its bucket maximum.
    NSLOT = NCORES * 8
    slot_expert = [-1] * NSLOT
    slot_tok = [np.empty(0, np.int64)] * NSLOT
    slot_gate = [np.empty(0, np.float32)] * NSLOT
    ranked = sorted(range(E), key=lambda e: -len(tok_by_e[e]))
    overflow = []   # (expert, tokens, gates) beyond the primary slot cap
    for r, e in enumerate(ranked):
        s = (r % NCORES) * 8 + (r // NCORES)
        cap = CAPS[r // NCORES]
        slot_expert[s] = e
        slot_tok[s] = tok_by_e[e][:cap]
        slot_gate[s] = gate_by_e[e][:cap]
        if len(tok_by_e[e]) > cap:
            overflow.append((e, tok_by_e[e][cap:], gate_by_e[e][cap:]))
    # worst overflow spills into the spare slot 63 (unit 7, cap CAPS[7]);
    # anything further goes to an exact host fallback (rare).
    host_fallback = []
    if overflow:
        overflow.sort(key=lambda t: -len(t[1]))
        e0, t0, g0 = overflow[0]
        cap63 = CAPS[7]
        slot_expert[63] = e0
        slot_tok[63] = t0[:cap63]
        slot_gate[63] = g0[:cap63]
        if len(t0) > cap63:
            host_fallback.append((e0, t0[cap63:], g0[cap63:]))
        for e, t, g in overflow[1:]:
            host_fallback.append((e, t, g))

    # ---------------- build per-core device inputs ----------------
    fp8 = wdt == "fp8"
    sx = S_X if fp8 else 1.0
    sw = S_W if fp8 else 1.0

    def q(a):
        # clip keeps accidental outliers out of fp8-inf territory (TRN
        # e4m3 max normal is 240); no-op for 16/32-bit working dtypes
        return (np.clip(a, -240.0, 240.0) if fp8 else a).astype(ndt)

    # x transposed + partition-tiled: xT_t[ko, p, t] = x[t, ko*128+p]
    xT_t = q(np.ascontiguousarray(xf.T) * sx).reshape(KO, 128, T)

    w1t_sh = q(_tile_w1(w1_shared[0]) * sw)
    w2t_sh = q(_tile_w2(w2_shared[0]) * sw)
    b1t_sh = b1_shared[0].reshape(KO, 128).T

    in_maps = []
    for c in range(NCORES):
        xg = np.zeros((UNITS, 128, KO, C), ndt)
        w1u = np.zeros((UNITS, N_W1C, 128, KO, W1CW), ndt)
        b1u = np.zeros((UNITS, 128, KO), np.float32)
        w2u = np.zeros((UNITS, N_W2C, 128, KO, W2CW), ndt)
        for u in range(8):
            s = c * 8 + u
            e = slot_expert[s]
            if e < 0 or len(slot_tok[s]) == 0:
                continue
            n = len(slot_tok[s])
            idx = np.zeros(C, np.int64)
            idx[:n] = slot_tok[s]
            xg[u] = xT_t[:, :, idx].swapaxes(0, 1)
            w1u[u] = q(_tile_w1(w1[e]) * sw)
            b1u[u] = b1[e].reshape(KO, 128).T
            w2u[u] = q(_tile_w2(w2[e]) * sw)
        # shared-expert unit
        xg[8] = xT_t[:, :, c * TSH:(c + 1) * TSH].swapaxes(0, 1)
        w1u[8] = w1t_sh
        b1u[8] = b1t_sh
        w2u[8] = w2t_sh
        in_maps.append({"xg": xg, "w1": w1u, "b1": b1u, "w2": w2u})

    # ---------------- run on 8 cores ----------------
    nc = _get_nc(wdt)
    res = run_bass_kernel_spmd(nc, in_maps, core_ids=list(range(NCORES)))
    outs = [r["out"] for r in res.results]   # [UNITS, 128, CM, HID] each

    # ---------------- host unshard / scatter ----------------
    # device output is transposed: outs[c][u][p, hk, c'] = y[c', hk*128+p]
    inv2 = INV_S2 if fp8 else 1.0   # undo the S_W scale baked into mm2

    def untile_y(o, n):
        y = o.transpose(1, 0, 2).reshape(HID, C)[:, :n].T.astype(np.float32)
        return y * inv2 if fp8 else y

    acc = np.zeros((T, HID), np.float32)     # shared + routed
    # shared expert (unit 8 on each core), gate 1, + b2_shared
    for c in range(NCORES):
        ys = untile_y(outs[c][8], TSH)
        acc[c * TSH:(c + 1) * TSH] = ys + b2_shared[0]
    # routed experts: gate * (y + b2), scattered by token
    for s in range(NCORES * 8):
        e = slot_expert[s]
        n = len(slot_tok[s])
        if e < 0 or n == 0:
            continue
        ye = untile_y(outs[s // 8][s % 8], n)
        # token indices are unique within one slot, so fancy += is safe
        acc[slot_tok[s]] += slot_gate[s][:, None] * (ye + b2[e][None, :])
    # exact host fallback for overflow beyond device capacity
    for e, toks, gs in host_fallback:
        h = _gelu_np(xf[toks] @ w1[e] + b1[e])
        acc[toks] += gs[:, None] * (h @ w2[e] + b2[e])

    return (acc + xf).reshape(B, S, HID).astype(np.float32)



# revision 15
# speedup vs baseline: 1.8370x; 1.0061x over previous
"""MoE (63 routed experts, top-7, 1 shared expert) Trainium2 Bass kernel.

Strategy (expert parallelism, per sharding hint):
  - Host: router matmul + softmax + top-k (tiny: 0.7 GFLOP vs 220 GFLOP of
    expert FFNs), token gather per expert.
  - Device (8 NeuronCores, SPMD): each core runs 9 "units" of identical
    shape: 8 routed-expert slots (64 slots globally = 63 experts + 1
    overflow slot) and 1 shared-expert slot over a 1/8 token slice.
    Each unit: h = gelu(XeT^T @ W1 + b1); y = gate * (h @ W2), with
    full-rate matmuls (float32r or bf16), GELU fused into the PSUM
    eviction on the scalar engine, gating fused into the PSUM eviction on
    the vector engine.  Weights are host-pretiled into chunk-contiguous
    layout so every DMA is a flat [128 x bytes] block.
  - Host: scatter-add gated expert outputs (+ gate*b2), add shared out,
    bias and residual.

Experts are assigned to slots by descending load rank with static per-unit
token capacities (CAPS); both matmul layers' free dim is the capacity, so
PE cost tracks actual expert load.  Overload spills into the spare 64th
slot and, beyond that, to an exact host-side FFN for the few excess
tokens.  Gating and b2 are applied on the host during the scatter.
"""

import os

import numpy as np

B, S, HID = 2, 2048, 1280
E = 63
I = 1280
TOP_K = 7
NCORES = 8
UNITS = 9          # 8 expert slots + 1 shared-expert slot
C = 512            # token capacity per expert slot
CM = C // 128      # 4
KO = HID // 128    # 10 contraction chunks
T = B * S          # 4096
TSH = T // NCORES  # 512 shared-expert tokens per core

W1CW = 256          # w1 chunk width along I (2 lhsT column groups)
W2CW = 256          # w2 chunk width along H (2 lhsT column groups)
N_W1C = I // W1CW   # 5
N_W2C = HID // W2CW  # 5

# Per-unit-index token capacities. Experts are assigned to slots by load
# rank (rank r -> core r%8, unit r//8), so unit j only ever sees the j-th
# bucket of the descending load distribution; caps track the observed
# bucket maxima (+2). Overflow goes to the spare slot 63 and, beyond
# that, to an exact host fallback, so an under-sized cap costs host time
# and a little accuracy headroom, never correctness.
CAPS = [512, 493, 477, 465, 453, 442, 434, 423, C]   # unit 8 = shared

# "f32r": fp32 data, full-rate float32r matmuls (most accurate).
# "bf16": bf16 weights+activations, fp32 accumulate (halves DMA traffic).
# "fp16": like bf16 but 4x finer mantissa; all values here are well within
#         fp16 range, so this is strictly more accurate at the same speed.
# "fp8":  e4m3 weights+activations with DoubleRow matmuls (0.5 cycles/row,
#         ~2x PE throughput).  Inputs are pre-scaled into e4m3's sweet spot
#         (S_X for x, S_W for both weight matrices); the mm1 descale is
#         folded into the GELU's input scale, the mm2 descale into the host
#         scatter.  fp16 device output halves the drain DMA.
WORK_DTYPE = os.environ.get("MOE_WDT", "fp8")

S_X = 16.0          # x -> fp8 scale
S_W = 64.0          # w1, w2 -> fp8 scale
INV_S1 = 1.0 / (S_X * S_W)   # PSUM descale before GELU (mm1)
INV_S2 = 1.0 / S_W           # host descale of mm2 output

_cache = {}


def _build_nc(wdt):
    import concourse.mybir as mybir
    import concourse.tile as tile
    from concourse import bacc

    f32 = mybir.dt.float32
    GELU = mybir.ActivationFunctionType.Gelu
    if os.environ.get("MOE_SIM_NOGELU"):      # CoreSim lacks Gelu; layout-
        GELU = mybir.ActivationFunctionType.Identity   # check runs use this
    fp8 = wdt == "fp8"
    DR = mybir.MatmulPerfMode.DoubleRow if fp8 else None
    if wdt == "f32r":
        mdt = mybir.dt.float32r
        ddt = f32    # dram dtype for weight/activation tensors
        odt = f32
        bufs = dict(xu=2, h1=2, w1c=3, w2c=3, ou=2)
    elif fp8:
        mdt = mybir.dt.float8e4
        ddt = mdt
        odt = mybir.dt.float16
        bufs = dict(xu=4, h1=3, w1c=6, w2c=4, ou=3)
    else:
        mdt = mybir.dt.float16 if wdt == "fp16" else mybir.dt.bfloat16
        ddt = mdt
        odt = f32
        bufs = dict(xu=3, h1=3, w1c=4, w2c=4, ou=2)

    nc = bacc.Bacc(None, target_bir_lowering=False)

    xg_d = nc.dram_tensor("xg", [UNITS, 128, KO, C], ddt, kind="ExternalInput")
    w1_d = nc.dram_tensor("w1", [UNITS, N_W1C, 128, KO, W1CW], ddt,
                          kind="ExternalInput")
    b1_d = nc.dram_tensor("b1", [UNITS, 128, KO], f32, kind="ExternalInput")
    w2_d = nc.dram_tensor("w2", [UNITS, N_W2C, 128, KO, W2CW], ddt,
                          kind="ExternalInput")
    # transposed output: out[u, p, hk, c] = y[token c, h = hk*128+p]
    out_d = nc.dram_tensor("out", [UNITS, 128, KO, C], odt, kind="ExternalOutput")

    def cast(ap):
        return ap.bitcast(mdt) if wdt == "f32r" else ap

    with tile.TileContext(nc) as tc:
        with tc.tile_pool(name="xg_p", bufs=bufs["xu"]) as xg_p, \
             tc.tile_pool(name="h1_p", bufs=bufs["h1"]) as h1_p, \
             tc.tile_pool(name="w1_p", bufs=bufs["w1c"]) as w1_p, \
             tc.tile_pool(name="w2_p", bufs=bufs["w2c"]) as w2_p, \
             tc.tile_pool(name="out_p", bufs=bufs["ou"]) as out_p, \
             tc.tile_pool(name="sm_p", bufs=3) as sm_p, \
             tc.tile_pool(name="ps1_p", bufs=3, space="PSUM") as ps1_p, \
             tc.tile_pool(name="ps2_p", bufs=4, space="PSUM") as ps2_p:

            # shared unit (heaviest) first, smallest-cap unit last: the
            # kernel tail scales with the final unit's capacity
            order = [UNITS - 1] + list(range(UNITS - 1)) if UNITS > 1 else [0]
            for idx, u in enumerate(order):
                CAP = CAPS[u]
                last_u = idx == len(order) - 1
                w1cs = {}
                # first w1 chunk ahead of everything else the unit needs
                w1cs[0] = w1_p.tile([128, KO, W1CW], mdt, tag="w1c", name="w1c")
                xu = xg_p.tile([128, KO, C], mdt, tag="xu")
                b1u = sm_p.tile([128, KO], f32, tag="b1u")
                if idx == 0:
                    # head-latency trim: land just the slices the first PSUM
                    # chain needs, then the rest
                    nc.sync.dma_start(w1cs[0][:, :2], cast(w1_d[u, 0, :, :2]))
                    nc.sync.dma_start(xu[:, :2, :CAP],
                                      cast(xg_d[u, :, :2, :CAP]))
                    nc.sync.dma_start(b1u[:], b1_d[u])
                    nc.sync.dma_start(w1cs[0][:, 2:], cast(w1_d[u, 0, :, 2:]))
                    nc.sync.dma_start(xu[:, 2:, :CAP],
                                      cast(xg_d[u, :, 2:, :CAP]))
                else:
                    nc.sync.dma_start(w1cs[0][:], cast(w1_d[u, 0]))
                    # split halves so the first matmuls can start sooner
                    nc.sync.dma_start(xu[:, :KO // 2, :CAP],
                                      cast(xg_d[u, :, :KO // 2, :CAP]))
                    nc.sync.dma_start(xu[:, KO // 2:, :CAP],
                                      cast(xg_d[u, :, KO // 2:, :CAP]))
                    nc.sync.dma_start(b1u[:], b1_d[u])

                h1 = h1_p.tile([128, KO, C], mdt, tag="h1")

                # ---- mm1: h1[i, c] = gelu(sum_h W1[h,i] * X^T[h,c] + b1[i])
                for ic in range(N_W1C):
                    if ic not in w1cs:
                        w1cs[ic] = w1_p.tile([128, KO, W1CW], mdt, tag="w1c", name="w1c")
                        nc.sync.dma_start(w1cs[ic][:], cast(w1_d[u, ic]))
                    w1c = w1cs[ic]
                    for s in range(W1CW // 128):
                        i_out = ic * (W1CW // 128) + s
                        ps = ps1_p.tile([128, C], f32, tag="ps1")
                        if fp8:
                            for k2 in range(KO // 2):
                                nc.tensor.matmul(
                                    ps[:, :CAP],
                                    w1c[:, 2 * k2:2 * k2 + 2, s * 128:(s + 1) * 128],
                                    xu[:, 2 * k2:2 * k2 + 2, :CAP],
                                    start=(k2 == 0),
                                    stop=(k2 == KO // 2 - 1),
                                    perf_mode=DR,
                                )
                        else:
                            for ko in range(KO):
                                nc.tensor.matmul(
                                    ps[:, :CAP],
                                    w1c[:, ko, s * 128:(s + 1) * 128],
                                    xu[:, ko, :CAP],
                                    start=(ko == 0),
                                    stop=(ko == KO - 1),
                                )
                        nc.scalar.activation(
                            h1[:, i_out, :CAP], ps[:, :CAP], GELU,
                            bias=b1u[:, i_out:i_out + 1],
                            scale=INV_S1 if fp8 else 1.0)

                # ---- mm2 (transposed): yT[h, c] = sum_i W2[i, h] * h1[i, c]
                # gating and b2 are applied on the host during scatter.
                oy = out_p.tile([128, KO, C], odt, tag="oy")
                for hcc in range(N_W2C):
                    w2c = w2_p.tile([128, KO, W2CW], mdt, tag="w2c")
                    nc.sync.dma_start(w2c[:], cast(w2_d[u, hcc]))
                    for s2 in range(W2CW // 128):
                        hk = hcc * (W2CW // 128) + s2
                        ps2 = ps2_p.tile([128, C], f32, tag="ps2")
                        if fp8:
                            for k2 in range(KO // 2):
                                nc.tensor.matmul(
                                    ps2[:, :CAP],
                                    w2c[:, 2 * k2:2 * k2 + 2, s2 * 128:(s2 + 1) * 128],
                                    h1[:, 2 * k2:2 * k2 + 2, :CAP],
                                    start=(k2 == 0),
                                    stop=(k2 == KO // 2 - 1),
                                    perf_mode=DR,
                                )
                        else:
                            for ko in range(KO):
                                nc.tensor.matmul(
                                    ps2[:, :CAP],
                                    w2c[:, ko, s2 * 128:(s2 + 1) * 128],
                                    h1[:, ko, :CAP],
                                    start=(ko == 0),
                                    stop=(ko == KO - 1),
                                )
                        nc.vector.tensor_copy(oy[:, hk, :CAP], ps2[:, :CAP])
                        # drain finished output rows early so the final DMA
                        # (and the kernel tail) stays small; a separate DMA
                        # queue (gpsimd) keeps drains from head-of-line
                        # blocking the next unit's input loads on sync.  The
                        # last unit drains on sync (empty by then, and its
                        # HWDGE path beats gpsimd's SWDGE at the exposed tail)
                        if hk % 2 == 1:
                            deng = nc.sync if last_u else nc.gpsimd
                            deng.dma_start(
                                out_d[u, :, hk - 1:hk + 1, :CAP],
                                oy[:, hk - 1:hk + 1, :CAP])

    nc.compile()
    return nc


def _get_nc(wdt):
    if wdt not in _cache:
        _cache[wdt] = _build_nc(wdt)
    return _cache[wdt]


def _np_wdt(wdt):
    if wdt == "bf16":
        import ml_dtypes
        return np.dtype(ml_dtypes.bfloat16)
    if wdt == "fp16":
        return np.dtype(np.float16)
    if wdt == "fp8":
        import ml_dtypes
        # TRN FP8_EXP4: bias 7, max normal 240 — ml_dtypes' IEEE e4m3
        return np.dtype(ml_dtypes.float8_e4m3)
    return np.dtype(np.float32)


def _gelu_np(v):
    from scipy.special import erf
    v = v.astype(np.float32)
    return (0.5 * v * (1.0 + erf(v / np.sqrt(2.0)))).astype(np.float32)


def _tile_w1(w):
    # [H, I] -> [N_W1C, 128, KO, W1CW] with w1t[ic, p, ko, j] = w[ko*128+p, ic*W1CW+j]
    return w.reshape(KO, 128, N_W1C, W1CW).transpose(2, 1, 0, 3)


def _tile_w2(w):
    # [I, H] -> [N_W2C, 128, KO, W2CW]
    return w.reshape(KO, 128, N_W2C, W2CW).transpose(2, 1, 0, 3)


def _ensure_axon_hooks_stub():
    """bass_utils' axon trace path imports antenv.axon_hooks, which this
    image lacks; provide a no-op stub so a BASS_TRACE-enabled environment
    degrades gracefully instead of crashing."""
    import sys
    import types
    try:
        import antenv.axon_hooks  # noqa: F401
        return
    except ImportError:
        pass
    try:
        import antenv
    except ImportError:
        return
    mod = types.ModuleType("antenv.axon_hooks")
    holder = [None]
    mod.set_axon_ntff_profile_hook = lambda h: holder.__setitem__(0, h)
    mod.get_axon_ntff_profile_hook = lambda: holder[0]
    sys.modules["antenv.axon_hooks"] = mod
    antenv.axon_hooks = mod


def kernel(x, w1_shared, b1_shared, w2_shared, b2_shared,
           router_w, router_b, w1, b1, w2, b2):
    _ensure_axon_hooks_stub()
    from concourse.bass_utils import run_bass_kernel_spmd

    wdt = WORK_DTYPE
    ndt = _np_wdt(wdt)

    x = np.asarray(x, np.float32)
    w1 = np.asarray(w1, np.float32)
    b1 = np.asarray(b1, np.float32)
    w2 = np.asarray(w2, np.float32)
    b2 = np.asarray(b2, np.float32)
    w1_shared = np.asarray(w1_shared, np.float32)
    b1_shared = np.asarray(b1_shared, np.float32)
    w2_shared = np.asarray(w2_shared, np.float32)
    b2_shared = np.asarray(b2_shared, np.float32)
    router_w = np.asarray(router_w, np.float32)
    router_b = np.asarray(router_b, np.float32)

    xf = x.reshape(T, HID)

    # ---------------- host routing ----------------
    logits = xf @ router_w + router_b
    m = logits.max(-1, keepdims=True)
    ex = np.exp(logits - m, dtype=np.float32)
    affin = ex / ex.sum(-1, keepdims=True, dtype=np.float32)
    order = np.argsort(-affin, axis=-1, kind="stable")[:, :TOP_K]   # [T, K]
    vals = np.take_along_axis(affin, order, axis=-1)                # [T, K]

    # group (token, gate) pairs by expert
    flat_e = order.ravel()
    flat_t = np.repeat(np.arange(T), TOP_K)
    flat_g = vals.ravel()
    sort = np.argsort(flat_e, kind="stable")
    se, st, sg = flat_e[sort], flat_t[sort], flat_g[sort]
    starts = np.searchsorted(se, np.arange(E + 1))
    tok_by_e = [st[starts[e]:starts[e + 1]] for e in range(E)]
    gate_by_e = [sg[starts[e]:starts[e + 1]] for e in range(E)]

    # slot table: 64 expert slots; slot s = core*8 + unit.  Experts are
    # assigned by descending load rank: rank r -> core r%8, unit r//8, so
    # every core gets one expert from each load bucket and unit j's static
    # capacity CAPS[j] covers its bucket maximum.
    NSLOT = NCORES * 8
    slot_expert = [-1] * NSLOT
    slot_tok = [np.empty(0, np.int64)] * NSLOT
    slot_gate = [np.empty(0, np.float32)] * NSLOT
    ranked = sorted(range(E), key=lambda e: -len(tok_by_e[e]))
    overflow = []   # (expert, tokens, gates) beyond the primary slot cap
    for r, e in enumerate(ranked):
        s = (r % NCORES) * 8 + (r // NCORES)
        cap = CAPS[r // NCORES]
        slot_expert[s] = e
        slot_tok[s] = tok_by_e[e][:cap]
        slot_gate[s] = gate_by_e[e][:cap]
        if len(tok_by_e[e]) > cap:
            overflow.append((e, tok_by_e[e][cap:], gate_by_e[e][cap:]))
    # worst overflow spills into the spare slot 63 (unit 7, cap CAPS[7]);
    # anything further goes to an exact host fallback (rare).
    host_fallback = []
    if overflow:
        overflow.sort(key=lambda t: -len(t[1]))
        e0, t0, g0 = overflow[0]
        cap63 = CAPS[7]
        slot_expert[63] = e0
        slot_tok[63] = t0[:cap63]
        slot_gate[63] = g0[:cap63]
        if len(t0) > cap63:
            host_fallback.append((e0, t0[cap63:], g0[cap63:]))
        for e, t, g in overflow[1:]:
            host_fallback.append((e, t, g))

    # ---------------- build per-core device inputs ----------------
    fp8 = wdt == "fp8"
    sx = S_X if fp8 else 1.0
    sw = S_W if fp8 else 1.0

    def q(a):
        # clip keeps accidental outliers out of fp8-inf territory (TRN
        # e4m3 max normal is 240); no-op for 16/32-bit working dtypes
        return (np.clip(a, -240.0, 240.0) if fp8 else a).astype(ndt)

    # x transposed + partition-tiled: xT_t[ko, p, t] = x[t, ko*128+p]
    xT_t = q(np.ascontiguousarray(xf.T) * sx).reshape(KO, 128, T)

    w1t_sh = q(_tile_w1(w1_shared[0]) * sw)
    w2t_sh = q(_tile_w2(w2_shared[0]) * sw)
    b1t_sh = b1_shared[0].reshape(KO, 128).T

    in_maps = []
    for c in range(NCORES):
        xg = np.zeros((UNITS, 128, KO, C), ndt)
        w1u = np.zeros((UNITS, N_W1C, 128, KO, W1CW), ndt)
        b1u = np.zeros((UNITS, 128, KO), np.float32)
        w2u = np.zeros((UNITS, N_W2C, 128, KO, W2CW), ndt)
        for u in range(8):
            s = c * 8 + u
            e = slot_expert[s]
            if e < 0 or len(slot_tok[s]) == 0:
                continue
            n = len(slot_tok[s])
            idx = np.zeros(C, np.int64)
            idx[:n] = slot_tok[s]
            xg[u] = xT_t[:, :, idx].swapaxes(0, 1)
            w1u[u] = q(_tile_w1(w1[e]) * sw)
            b1u[u] = b1[e].reshape(KO, 128).T
            w2u[u] = q(_tile_w2(w2[e]) * sw)
        # shared-expert unit
        xg[8] = xT_t[:, :, c * TSH:(c + 1) * TSH].swapaxes(0, 1)
        w1u[8] = w1t_sh
        b1u[8] = b1t_sh
        w2u[8] = w2t_sh
        in_maps.append({"xg": xg, "w1": w1u, "b1": b1u, "w2": w2u})

    # ---------------- run on 8 cores ----------------
    nc = _get_nc(wdt)
    res = run_bass_kernel_spmd(nc, in_maps, core_ids=list(range(NCORES)))
    outs = [r["out"] for r in res.results]   # [UNITS, 128, CM, HID] each

    # ---------------- host unshard / scatter ----------------
    # device output is transposed: outs[c][u][p, hk, c'] = y[c', hk*128+p]
    inv2 = INV_S2 if fp8 else 1.0   # undo the S_W scale baked into mm2

    def untile_y(o, n):
        y = o.transpose(1, 0, 2).reshape(HID, C)[:, :n].T.astype(np.float32)
        return y * inv2 if fp8 else y

    acc = np.zeros((T, HID), np.float32)     # shared + routed
    # shared expert (unit 8 on each core), gate 1, + b2_shared
    for c in range(NCORES):
        ys = untile_y(outs[c][8], TSH)
        acc[c * TSH:(c + 1) * TSH] = ys + b2_shared[0]
    # routed experts: gate * (y + b2), scattered by token
    for s in range(NCORES * 8):
        e = slot_expert[s]
        n = len(slot_tok[s])
        if e < 0 or n == 0:
            continue
        ye = untile_y(outs[s // 8][s % 8], n)
        # token indices are unique within one slot, so fancy += is safe
        acc[slot_tok[s]] += slot_gate[s][:, None] * (ye + b2[e][None, :])
    # exact host fallback for overflow beyond device capacity
    for e, toks, gs in host_fallback:
        h = _gelu_np(xf[toks] @ w1[e] + b1[e])
        acc[toks] += gs[:, None] * (h @ w2[e] + b2[e])

    return (acc + xf).reshape(B, S, HID).astype(np.float32)



# revision 16
# speedup vs baseline: 1.8851x; 1.0262x over previous
"""MoE (63 routed experts, top-7, 1 shared expert) Trainium2 Bass kernel.

Strategy (expert parallelism, per sharding hint):
  - Host: router matmul + softmax + top-k (tiny: 0.7 GFLOP vs 220 GFLOP of
    expert FFNs), token gather per expert.
  - Device (8 NeuronCores, SPMD): each core runs 9 "units" of identical
    shape: 8 routed-expert slots (64 slots globally = 63 experts + 1
    overflow slot) and 1 shared-expert slot over a 1/8 token slice.
    Each unit: h = gelu(XeT^T @ W1 + b1); y = gate * (h @ W2), with
    full-rate matmuls (float32r or bf16), GELU fused into the PSUM
    eviction on the scalar engine, gating fused into the PSUM eviction on
    the vector engine.  Weights are host-pretiled into chunk-contiguous
    layout so every DMA is a flat [128 x bytes] block.
  - Host: scatter-add gated expert outputs (+ gate*b2), add shared out,
    bias and residual.

Experts are assigned to slots by descending load rank with static per-unit
token capacities (CAPS); both matmul layers' free dim is the capacity, so
PE cost tracks actual expert load.  Overload spills into the spare 64th
slot and, beyond that, to an exact host-side FFN for the few excess
tokens.  Gating and b2 are applied on the host during the scatter.
"""

import os

import numpy as np

B, S, HID = 2, 2048, 1280
E = 63
I = 1280
TOP_K = 7
NCORES = 8
UNITS = 9          # 8 expert slots + 1 shared-expert slot
C = 512            # token capacity per expert slot
CM = C // 128      # 4
KO = HID // 128    # 10 contraction chunks
T = B * S          # 4096
TSH = T // NCORES  # 512 shared-expert tokens per core

W1CW = 256          # w1 chunk width along I (2 lhsT column groups)
W2CW = 256          # w2 chunk width along H (2 lhsT column groups)
N_W1C = I // W1CW   # 5
N_W2C = HID // W2CW  # 5

# Per-unit-index token capacities. Experts are assigned to slots by load
# rank (rank r -> core r%8, unit r//8), so unit j only ever sees the j-th
# bucket of the descending load distribution; caps track the observed
# bucket maxima (+2). Overflow goes to the spare slot 63 and, beyond
# that, to an exact host fallback, so an under-sized cap costs host time
# and a little accuracy headroom, never correctness.
CAPS = [512, 493, 477, 465, 453, 442, 434, 423, C]   # unit 8 = shared

# "f32r": fp32 data, full-rate float32r matmuls (most accurate).
# "bf16": bf16 weights+activations, fp32 accumulate (halves DMA traffic).
# "fp16": like bf16 but 4x finer mantissa; all values here are well within
#         fp16 range, so this is strictly more accurate at the same speed.
# "fp8":  e4m3 weights+activations with DoubleRow matmuls (0.5 cycles/row,
#         ~2x PE throughput).  Inputs are pre-scaled into e4m3's sweet spot
#         (S_X for x, S_W for both weight matrices); the mm1 descale is
#         folded into the GELU's input scale, the mm2 descale into the host
#         scatter.  fp16 device output halves the drain DMA.
WORK_DTYPE = os.environ.get("MOE_WDT", "fp8")

S_X = 16.0          # x -> fp8 scale
S_W = 64.0          # w1, w2 -> fp8 scale
INV_S1 = 1.0 / (S_X * S_W)   # PSUM descale before GELU (mm1)
INV_S2 = 1.0 / S_W           # host descale of mm2 output

_cache = {}


def _build_nc(wdt):
    import concourse.mybir as mybir
    import concourse.tile as tile
    from concourse import bacc

    f32 = mybir.dt.float32
    GELU = mybir.ActivationFunctionType.Gelu
    if os.environ.get("MOE_SIM_NOGELU"):      # CoreSim lacks Gelu; layout-
        GELU = mybir.ActivationFunctionType.Identity   # check runs use this
    fp8 = wdt == "fp8"
    DR = mybir.MatmulPerfMode.DoubleRow if fp8 else None
    if wdt == "f32r":
        mdt = mybir.dt.float32r
        ddt = f32    # dram dtype for weight/activation tensors
        odt = f32
        bufs = dict(xu=2, h1=2, w1c=3, w2c=3, ou=2)
    elif fp8:
        mdt = mybir.dt.float8e4
        ddt = mdt
        odt = mybir.dt.float16
        bufs = dict(xu=4, h1=3, w1c=6, w2c=4, ou=3)
    else:
        mdt = mybir.dt.float16 if wdt == "fp16" else mybir.dt.bfloat16
        ddt = mdt
        odt = f32
        bufs = dict(xu=3, h1=3, w1c=4, w2c=4, ou=2)

    nc = bacc.Bacc(None, target_bir_lowering=False)

    xg_d = nc.dram_tensor("xg", [UNITS, 128, KO, C], ddt, kind="ExternalInput")
    w1_d = nc.dram_tensor("w1", [UNITS, N_W1C, 128, KO, W1CW], ddt,
                          kind="ExternalInput")
    b1_d = nc.dram_tensor("b1", [UNITS, 128, KO], f32, kind="ExternalInput")
    w2_d = nc.dram_tensor("w2", [UNITS, N_W2C, 128, KO, W2CW], ddt,
                          kind="ExternalInput")
    # transposed output: out[u, p, hk, c] = y[token c, h = hk*128+p]
    out_d = nc.dram_tensor("out", [UNITS, 128, KO, C], odt, kind="ExternalOutput")

    def cast(ap):
        return ap.bitcast(mdt) if wdt == "f32r" else ap

    with tile.TileContext(nc) as tc:
        with tc.tile_pool(name="xg_p", bufs=bufs["xu"]) as xg_p, \
             tc.tile_pool(name="h1_p", bufs=bufs["h1"]) as h1_p, \
             tc.tile_pool(name="w1_p", bufs=bufs["w1c"]) as w1_p, \
             tc.tile_pool(name="w2_p", bufs=bufs["w2c"]) as w2_p, \
             tc.tile_pool(name="out_p", bufs=bufs["ou"]) as out_p, \
             tc.tile_pool(name="sm_p", bufs=3) as sm_p, \
             tc.tile_pool(name="ps1_p", bufs=3, space="PSUM") as ps1_p, \
             tc.tile_pool(name="ps2_p", bufs=4, space="PSUM") as ps2_p:

            # shared unit (heaviest) first, smallest-cap unit last: the
            # kernel tail scales with the final unit's capacity
            order = [UNITS - 1] + list(range(UNITS - 1)) if UNITS > 1 else [0]
            for idx, u in enumerate(order):
                CAP = CAPS[u]
                last_u = idx == len(order) - 1
                w1cs = {}
                # first w1 chunk ahead of everything else the unit needs
                w1cs[0] = w1_p.tile([128, KO, W1CW], mdt, tag="w1c", name="w1c")
                xu = xg_p.tile([128, KO, C], mdt, tag="xu")
                b1u = sm_p.tile([128, KO], f32, tag="b1u")
                if idx == 0:
                    # head-latency trim: land just the slices the first PSUM
                    # chain needs, then the rest
                    nc.sync.dma_start(w1cs[0][:, :2], cast(w1_d[u, 0, :, :2]))
                    nc.sync.dma_start(xu[:, :2, :CAP],
                                      cast(xg_d[u, :, :2, :CAP]))
                    nc.sync.dma_start(b1u[:], b1_d[u])
                    nc.sync.dma_start(w1cs[0][:, 2:], cast(w1_d[u, 0, :, 2:]))
                    nc.sync.dma_start(xu[:, 2:, :CAP],
                                      cast(xg_d[u, :, 2:, :CAP]))
                else:
                    nc.sync.dma_start(w1cs[0][:], cast(w1_d[u, 0]))
                    # split halves so the first matmuls can start sooner
                    nc.sync.dma_start(xu[:, :KO // 2, :CAP],
                                      cast(xg_d[u, :, :KO // 2, :CAP]))
                    nc.sync.dma_start(xu[:, KO // 2:, :CAP],
                                      cast(xg_d[u, :, KO // 2:, :CAP]))
                    nc.sync.dma_start(b1u[:], b1_d[u])

                h1 = h1_p.tile([128, KO, C], mdt, tag="h1")

                # ---- mm1: h1[i, c] = gelu(sum_h W1[h,i] * X^T[h,c] + b1[i])
                for ic in range(N_W1C):
                    if ic not in w1cs:
                        w1cs[ic] = w1_p.tile([128, KO, W1CW], mdt, tag="w1c", name="w1c")
                        nc.sync.dma_start(w1cs[ic][:], cast(w1_d[u, ic]))
                    w1c = w1cs[ic]
                    for s in range(W1CW // 128):
                        i_out = ic * (W1CW // 128) + s
                        ps = ps1_p.tile([128, C], f32, tag="ps1")
                        if fp8:
                            for k2 in range(KO // 2):
                                nc.tensor.matmul(
                                    ps[:, :CAP],
                                    w1c[:, 2 * k2:2 * k2 + 2, s * 128:(s + 1) * 128],
                                    xu[:, 2 * k2:2 * k2 + 2, :CAP],
                                    start=(k2 == 0),
                                    stop=(k2 == KO // 2 - 1),
                                    perf_mode=DR,
                                )
                        else:
                            for ko in range(KO):
                                nc.tensor.matmul(
                                    ps[:, :CAP],
                                    w1c[:, ko, s * 128:(s + 1) * 128],
                                    xu[:, ko, :CAP],
                                    start=(ko == 0),
                                    stop=(ko == KO - 1),
                                )
                        nc.scalar.activation(
                            h1[:, i_out, :CAP], ps[:, :CAP], GELU,
                            bias=b1u[:, i_out:i_out + 1],
                            scale=INV_S1 if fp8 else 1.0)

                # ---- mm2 (transposed): yT[h, c] = sum_i W2[i, h] * h1[i, c]
                # gating and b2 are applied on the host during scatter.
                oy = out_p.tile([128, KO, C], odt, tag="oy")
                for hcc in range(N_W2C):
                    w2c = w2_p.tile([128, KO, W2CW], mdt, tag="w2c")
                    nc.sync.dma_start(w2c[:], cast(w2_d[u, hcc]))
                    for s2 in range(W2CW // 128):
                        hk = hcc * (W2CW // 128) + s2
                        ps2 = ps2_p.tile([128, C], f32, tag="ps2")
                        if fp8:
                            for k2 in range(KO // 2):
                                nc.tensor.matmul(
                                    ps2[:, :CAP],
                                    w2c[:, 2 * k2:2 * k2 + 2, s2 * 128:(s2 + 1) * 128],
                                    h1[:, 2 * k2:2 * k2 + 2, :CAP],
                                    start=(k2 == 0),
                                    stop=(k2 == KO // 2 - 1),
                                    perf_mode=DR,
                                )
                        else:
                            for ko in range(KO):
                                nc.tensor.matmul(
                                    ps2[:, :CAP],
                                    w2c[:, ko, s2 * 128:(s2 + 1) * 128],
                                    h1[:, ko, :CAP],
                                    start=(ko == 0),
                                    stop=(ko == KO - 1),
                                )
                        nc.vector.tensor_copy(oy[:, hk, :CAP], ps2[:, :CAP])
                        # drain finished output rows early so the final DMA
                        # (and the kernel tail) stays small; a separate DMA
                        # queue (gpsimd) keeps drains from head-of-line
                        # blocking the next unit's input loads on sync.  The
                        # last unit drains on sync (empty by then, and its
                        # HWDGE path beats gpsimd's SWDGE at the exposed tail)
                        if last_u:
                            # per-hk drains at the tail: halves the final
                            # exposed transfer; sync's HWDGE path is idle
                            # (no loads left) and faster than SWDGE
                            nc.sync.dma_start(out_d[u, :, hk:hk + 1, :CAP],
                                              oy[:, hk:hk + 1, :CAP])
                        elif hk % 2 == 1:
                            nc.gpsimd.dma_start(
                                out_d[u, :, hk - 1:hk + 1, :CAP],
                                oy[:, hk - 1:hk + 1, :CAP])

    nc.compile()
    return nc


def _get_nc(wdt):
    if wdt not in _cache:
        _cache[wdt] = _build_nc(wdt)
    return _cache[wdt]


def _np_wdt(wdt):
    if wdt == "bf16":
        import ml_dtypes
        return np.dtype(ml_dtypes.bfloat16)
    if wdt == "fp16":
        return np.dtype(np.float16)
    if wdt == "fp8":
        import ml_dtypes
        # TRN FP8_EXP4: bias 7, max normal 240 — ml_dtypes' IEEE e4m3
        return np.dtype(ml_dtypes.float8_e4m3)
    return np.dtype(np.float32)


def _gelu_np(v):
    from scipy.special import erf
    v = v.astype(np.float32)
    return (0.5 * v * (1.0 + erf(v / np.sqrt(2.0)))).astype(np.float32)


def _tile_w1(w):
    # [H, I] -> [N_W1C, 128, KO, W1CW] with w1t[ic, p, ko, j] = w[ko*128+p, ic*W1CW+j]
    return w.reshape(KO, 128, N_W1C, W1CW).transpose(2, 1, 0, 3)


def _tile_w2(w):
    # [I, H] -> [N_W2C, 128, KO, W2CW]
    return w.reshape(KO, 128, N_W2C, W2CW).transpose(2, 1, 0, 3)


def _ensure_axon_hooks_stub():
    """bass_utils' axon trace path imports antenv.axon_hooks, which this
    image lacks; provide a no-op stub so a BASS_TRACE-enabled environment
    degrades gracefully instead of crashing."""
    import sys
    import types
    try:
        import antenv.axon_hooks  # noqa: F401
        return
    except ImportError:
        pass
    try:
        import antenv
    except ImportError:
        return
    mod = types.ModuleType("antenv.axon_hooks")
    holder = [None]
    mod.set_axon_ntff_profile_hook = lambda h: holder.__setitem__(0, h)
    mod.get_axon_ntff_profile_hook = lambda: holder[0]
    sys.modules["antenv.axon_hooks"] = mod
    antenv.axon_hooks = mod


def kernel(x, w1_shared, b1_shared, w2_shared, b2_shared,
           router_w, router_b, w1, b1, w2, b2):
    _ensure_axon_hooks_stub()
    from concourse.bass_utils import run_bass_kernel_spmd

    wdt = WORK_DTYPE
    ndt = _np_wdt(wdt)

    x = np.asarray(x, np.float32)
    w1 = np.asarray(w1, np.float32)
    b1 = np.asarray(b1, np.float32)
    w2 = np.asarray(w2, np.float32)
    b2 = np.asarray(b2, np.float32)
    w1_shared = np.asarray(w1_shared, np.float32)
    b1_shared = np.asarray(b1_shared, np.float32)
    w2_shared = np.asarray(w2_shared, np.float32)
    b2_shared = np.asarray(b2_shared, np.float32)
    router_w = np.asarray(router_w, np.float32)
    router_b = np.asarray(router_b, np.float32)

    xf = x.reshape(T, HID)

    # ---------------- host routing ----------------
    logits = xf @ router_w + router_b
    m = logits.max(-1, keepdims=True)
    ex = np.exp(logits - m, dtype=np.float32)
    affin = ex / ex.sum(-1, keepdims=True, dtype=np.float32)
    order = np.argsort(-affin, axis=-1, kind="stable")[:, :TOP_K]   # [T, K]
    vals = np.take_along_axis(affin, order, axis=-1)                # [T, K]

    # group (token, gate) pairs by expert
    flat_e = order.ravel()
    flat_t = np.repeat(np.arange(T), TOP_K)
    flat_g = vals.ravel()
    sort = np.argsort(flat_e, kind="stable")
    se, st, sg = flat_e[sort], flat_t[sort], flat_g[sort]
    starts = np.searchsorted(se, np.arange(E + 1))
    tok_by_e = [st[starts[e]:starts[e + 1]] for e in range(E)]
    gate_by_e = [sg[starts[e]:starts[e + 1]] for e in range(E)]

    # slot table: 64 expert slots; slot s = core*8 + unit.  Experts are
    # assigned by descending load rank: rank r -> core r%8, unit r//8, so
    # every core gets one expert from each load bucket and unit j's static
    # capacity CAPS[j] covers its bucket maximum.
    NSLOT = NCORES * 8
    slot_expert = [-1] * NSLOT
    slot_tok = [np.empty(0, np.int64)] * NSLOT
    slot_gate = [np.empty(0, np.float32)] * NSLOT
    ranked = sorted(range(E), key=lambda e: -len(tok_by_e[e]))
    overflow = []   # (expert, tokens, gates) beyond the primary slot cap
    for r, e in enumerate(ranked):
        s = (r % NCORES) * 8 + (r // NCORES)
        cap = CAPS[r // NCORES]
        slot_expert[s] = e
        slot_tok[s] = tok_by_e[e][:cap]
        slot_gate[s] = gate_by_e[e][:cap]
        if len(tok_by_e[e]) > cap:
            overflow.append((e, tok_by_e[e][cap:], gate_by_e[e][cap:]))
    # worst overflow spills into the spare slot 63 (unit 7, cap CAPS[7]);
    # anything further goes to an exact host fallback (rare).
    host_fallback = []
    if overflow:
        overflow.sort(key=lambda t: -len(t[1]))
        e0, t0, g0 = overflow[0]
        cap63 = CAPS[7]
        slot_expert[63] = e0
        slot_tok[63] = t0[:cap63]
        slot_gate[63] = g0[:cap63]
        if len(t0) > cap63:
            host_fallback.append((e0, t0[cap63:], g0[cap63:]))
        for e, t, g in overflow[1:]:
            host_fallback.append((e, t, g))

    # ---------------- build per-core device inputs ----------------
    fp8 = wdt == "fp8"
    sx = S_X if fp8 else 1.0
    sw = S_W if fp8 else 1.0

    def q(a):
        # clip keeps accidental outliers out of fp8-inf territory (TRN
        # e4m3 max normal is 240); no-op for 16/32-bit working dtypes
        return (np.clip(a, -240.0, 240.0) if fp8 else a).astype(ndt)

    # x transposed + partition-tiled: xT_t[ko, p, t] = x[t, ko*128+p]
    xT_t = q(np.ascontiguousarray(xf.T) * sx).reshape(KO, 128, T)

    w1t_sh = q(_tile_w1(w1_shared[0]) * sw)
    w2t_sh = q(_tile_w2(w2_shared[0]) * sw)
    b1t_sh = b1_shared[0].reshape(KO, 128).T

    in_maps = []
    for c in range(NCORES):
        xg = np.zeros((UNITS, 128, KO, C), ndt)
        w1u = np.zeros((UNITS, N_W1C, 128, KO, W1CW), ndt)
        b1u = np.zeros((UNITS, 128, KO), np.float32)
        w2u = np.zeros((UNITS, N_W2C, 128, KO, W2CW), ndt)
        for u in range(8):
            s = c * 8 + u
            e = slot_expert[s]
            if e < 0 or len(slot_tok[s]) == 0:
                continue
            n = len(slot_tok[s])
            idx = np.zeros(C, np.int64)
            idx[:n] = slot_tok[s]
            xg[u] = xT_t[:, :, idx].swapaxes(0, 1)
            w1u[u] = q(_tile_w1(w1[e]) * sw)
            b1u[u] = b1[e].reshape(KO, 128).T
            w2u[u] = q(_tile_w2(w2[e]) * sw)
        # shared-expert unit
        xg[8] = xT_t[:, :, c * TSH:(c + 1) * TSH].swapaxes(0, 1)
        w1u[8] = w1t_sh
        b1u[8] = b1t_sh
        w2u[8] = w2t_sh
        in_maps.append({"xg": xg, "w1": w1u, "b1": b1u, "w2": w2u})

    # ---------------- run on 8 cores ----------------
    nc = _get_nc(wdt)
    res = run_bass_kernel_spmd(nc, in_maps, core_ids=list(range(NCORES)))
    outs = [r["out"] for r in res.results]   # [UNITS, 128, CM, HID] each

    # ---------------- host unshard / scatter ----------------
    # device output is transposed: outs[c][u][p, hk, c'] = y[c', hk*128+p]
    inv2 = INV_S2 if fp8 else 1.0   # undo the S_W scale baked into mm2

    def untile_y(o, n):
        y = o.transpose(1, 0, 2).reshape(HID, C)[:, :n].T.astype(np.float32)
        return y * inv2 if fp8 else y

    acc = np.zeros((T, HID), np.float32)     # shared + routed
    # shared expert (unit 8 on each core), gate 1, + b2_shared
    for c in range(NCORES):
        ys = untile_y(outs[c][8], TSH)
        acc[c * TSH:(c + 1) * TSH] = ys + b2_shared[0]
    # routed experts: gate * (y + b2), scattered by token
    for s in range(NCORES * 8):
        e = slot_expert[s]
        n = len(slot_tok[s])
        if e < 0 or n == 0:
            continue
        ye = untile_y(outs[s // 8][s % 8], n)
        # token indices are unique within one slot, so fancy += is safe
        acc[slot_tok[s]] += slot_gate[s][:, None] * (ye + b2[e][None, :])
    # exact host fallback for overflow beyond device capacity
    for e, toks, gs in host_fallback:
        h = _gelu_np(xf[toks] @ w1[e] + b1[e])
        acc[toks] += gs[:, None] * (h @ w2[e] + b2[e])

    return (acc + xf).reshape(B, S, HID).astype(np.float32)

